# revision 86
# baseline (speedup 1.0000x reference)
"""Trainium2 Bass kernel for 16-head causal attention with relative position
bias (B=4, S=2048, D=1024, H=16, HD=64), distributed over 8 NeuronCores.

Sharding: tensor-parallel over heads - each core owns 2 heads end-to-end
(QKV projection column-sharded, attention, then an on-device AllToAll
re-shards by tokens so each core runs the output projection for a disjoint
1024-token slice). Host only slices weights / concatenates output slices.

v2 restructure (vs the first working kernel):
  - Blocks reordered: all hf=1 (odd-qmb) attention first, so the first
    AllToAll fires at ~60% of attention and overlaps the whole hf=0 phase.
  - The program is JIT-specialized on the actual key-padding lengths
    (read from the inputs at kernel() time): k-tiles and q-columns beyond
    a sequence's length are skipped exactly (their reference contribution
    is identically zero); QKV projection token tiles are trimmed to the
    128-rounded length (~20% less attention work for this seed).
  - V is projected in [vd, tok] orientation with 512-wide matmuls and
    PE-transposed back (the old [tok, vd] form needed 4x the matmul
    instructions at 128-free each).
  - Softmax-normalization broadcast matmul runs in bf16 (the fp32
    LOW_HIGH pairs cost 2.3us each on hw); all 16 Z-reciprocals are
    batched into one DVE pass.
  - Hard-won scheduling rules for the collectives (each violation cost
    15-60us of pipeline stall on hw, invisible in the cost model):
      * A collective's descriptors ride the sync(SP) DGE; ANY queued DMA
        that waits on the collective's semaphore starves the collective
        itself (deadlock until partial drain). All receive-side DMAs
        therefore ride the gpsimd queue, which is idle-by-construction
        while a collective is in flight.
      * No gpsimd compute may be queued behind a collective trigger
        (the queue head blocks until the collective completes), so the
        hf=0 softmax multiplies run vector-only.
      * The tile scheduler hoists collective-dependent work into the
        attention-phase engine streams (its cost model underestimates
        real a2a time 2-4x); the hf=1 Z-normalization is gated on a
        dummy value written into the last attention block's output tile
        so it cannot be hoisted above the end of attention.
  - Consolidated input layouts (packed wqkv, one constant vector) to kill
    256B/4B-descriptor DMAs that serialized ~15us of startup.

Compute dtype: bf16 matmul inputs, fp32 logits/accumulation.
Rel. error vs fp32 reference: 6.1e-3 (bit-identical across runs; the
original kernel showed trace-dependent corruption up to 2.2e-2).
"""

from contextlib import ExitStack
from itertools import chain

import numpy as np
import ml_dtypes

import concourse.bass as bass
import concourse.mybir as mybir
from concourse import bacc
from concourse.tile import TileContext
from concourse.bass_utils import run_bass_kernel_spmd

B, S, D, H = 4, 2048, 1024, 16
HD = D // H                  # 64
NC_ = 8                      # cores
HPC = H // NC_               # 2 heads per core
T = B * S                    # 8192 tokens
TPC = T // NC_               # 1024 tokens per core (out-proj shard)
NEG = -1e9
FP32 = mybir.dt.float32
BF16 = mybir.dt.bfloat16

KTILES = S // 128            # 16 k-tiles per sequence
IDENT = mybir.ActivationFunctionType.Identity
EXP = mybir.ActivationFunctionType.Exp

HF1 = [(0, 3), (0, 1), (1, 3), (1, 1), (2, 3), (2, 1), (3, 3), (3, 1)]
HF0 = [(0, 0), (0, 2), (1, 0), (1, 2), (2, 0), (2, 2), (3, 0), (3, 2)]
KA_BRIDGE = 0    # keepalive matmuls bridging the a2a(0) wait (throttled
                 # PE gains nothing from p-state keepalives; keep 0)
DB = 6           # bias-tile DMA prefetch depth (absorbs sync-queue stalls)


def build_program(lens) -> bass.Bass:
    """Build the (identical-on-every-core) SPMD Bass program, specialized
    to the per-batch valid lengths `lens` (from key_padding_mask)."""
    lens = [int(l) for l in lens]
    KTC = [(l + 127) // 128 for l in lens]           # valid k-tiles per b
    L128 = [ktc * 128 for ktc in KTC]                # 128-rounded lengths
    # qkv token-tile widths (128-rounded so K/V rows stay finite)
    TW = [[max(0, min(512, L128[b] - tb * 512)) for tb in range(4)]
          for b in range(B)]
    # exact q widths per 512-block
    QW = [[max(0, min(512, lens[b] - qmb * 512)) for qmb in range(4)]
          for b in range(B)]

    nc = bacc.Bacc(num_devices=NC_)

    # ---- I/O ----
    xT = nc.dram_tensor("xT", [D, T], BF16, kind="ExternalInput")
    # wqkv pre-arranged on host: [p, (q|k|v), fo, m] so each partition row
    # is one contiguous 2KB-per-weight run (256B descriptors cost 2x)
    wqkv = nc.dram_tensor("wqkv", [128, 3, 8, 128], BF16,
                          kind="ExternalInput")
    # cvec: col 0 bq, 1 bk, 2 bv, 3.. pc (B*KTILES)
    cvec = nc.dram_tensor("cvec", [128, 3 + B * KTILES], FP32,
                          kind="ExternalInput")
    # transposed multiplicative bias: ebT[h, k, q] =
    #   exp(rel_bias[h, q, k] + causal[q, k])  (exactly 0 where masked)
    ebT = nc.dram_tensor("ebT", [HPC, S, S], BF16, kind="ExternalInput")
    wout = nc.dram_tensor("wout", [D, D], BF16, kind="ExternalInput")
    boutb = nc.dram_tensor("boutb", [128, D], FP32, kind="ExternalInput")
    npad = nc.dram_tensor("npad", [128, TPC // 128], FP32, kind="ExternalInput")
    # sel2d[r, i, p] = 1 iff r == 2*i + (p >= 64): per-source Z broadcaster
    sel2d = nc.dram_tensor("sel2d", [2 * NC_, NC_, 128], BF16,
                           kind="ExternalInput")
    idn = nc.dram_tensor("idn", [128, 128], BF16, kind="ExternalInput")
    out = nc.dram_tensor("out", [TPC, D], FP32, kind="ExternalOutput")

    with TileContext(nc) as tc:
        with tc.tile_pool(name="const", bufs=1) as const, \
             tc.tile_pool(name="dram", bufs=1, space="DRAM") as dpool, \
             tc.tile_pool(name="big", bufs=1) as big, \
             tc.tile_pool(name="xp", bufs=2) as xp, \
             tc.tile_pool(name="vtp", bufs=2) as vtp, \
             tc.tile_pool(name="bcache", bufs=1) as bcache, \
             tc.tile_pool(name="bstream", bufs=8) as bstream, \
             tc.tile_pool(name="esp", bufs=3) as esp, \
             tc.tile_pool(name="ptp", bufs=3) as ptp, \
             tc.tile_pool(name="ap_", bufs=2) as ap_, \
             tc.tile_pool(name="recvp", bufs=1) as recvp, \
             tc.tile_pool(name="rzp", bufs=2) as rzp, \
             tc.tile_pool(name="op_", bufs=2) as op_:

            # ---- constants: first-use loads split across all 3 DGE queues
            # preload the tb=3 x tile of b=0 (first consumed: block (0,3))
            xt0 = xp.tile([128, 8, 512], BF16, tag="xt", name="xt0")
            xT_r0 = xT.rearrange("(fo p) t -> p fo t", p=128)
            tw0 = TW[0][3]
            wqkv_sb = const.tile([128, 3, 8, 128], BF16, tag="wqkv")
            nc.gpsimd.dma_start(wqkv_sb, wqkv[:])
            if tw0 > 0:
                nc.sync.dma_start(xt0[:, :, :tw0],
                                  xT_r0[:, :, 1536:1536 + tw0])
            wq_sb, wk_sb, wv_sb = (wqkv_sb[:, i] for i in range(3))
            cvec_sb = const.tile([128, 3 + B * KTILES], FP32, tag="cvec")
            nc.sync.dma_start(cvec_sb, cvec[:])
            bq_sb = cvec_sb[:, 0:1]
            bk_sb = cvec_sb[:, 1:2]
            bv_sb = cvec_sb[:, 2:3]

            def pc_col(b, kc):
                i = 3 + b * KTILES + kc
                return cvec_sb[:, i:i + 1]
            # selector for the per-head Z broadcast: row h -> out rows h*64..
            sel2 = const.tile([2 * NC_, NC_, 128], BF16, tag="sel2")
            nc.sync.dma_start(sel2, sel2d[:])
            idn_sb = const.tile([128, 128], BF16, tag="idn")
            nc.sync.dma_start(idn_sb, idn[:])

            # ---- internal DRAM for the AllToAlls ----
            # channels: "1" = whole hf=1 half (fully overlapped by hf=0
            # attention); "0a"/"0b" = column-halves of hf=0, pipelined so
            # the first half's projection overlaps the second's transfer.
            CHW = {"1": TPC // 2, "0": TPC // 2}
            CH_T0 = {"1": 4, "0": 0}   # first out token-tile
            a2a_in = {c: dpool.tile([NC_, 65, HPC, w], BF16,
                                    tag=f"a2a_in{c}", name=f"a2a_in{c}")
                      for c, w in CHW.items()}
            a2a_out = {c: dpool.tile([NC_, 65, HPC, w], BF16,
                                     tag=f"a2a_out{c}", name=f"a2a_out{c}")
                       for c, w in CHW.items()}

            # ---- persistent per-b intermediates ----
            # QT/KT: [2*HD qdims (h0 0:64, h1 64:128), S tokens]
            QT = [big.tile([128, S], BF16, tag=f"QT{b}", name=f"QT{b}")
                  for b in range(B)]
            KT = [big.tile([128, S], BF16, tag=f"KT{b}", name=f"KT{b}")
                  for b in range(B)]
            # V: [128 token-part, 16 token-chunks, 130]:
            #   cols 0:64 head0, 64 ones, 65:129 head1, 129 ones
            V = [big.tile([128, KTILES, 130], BF16, tag=f"V{b}", name=f"V{b}")
                 for b in range(B)]
            for b in range(B):
                nc.gpsimd.memset(V[b][:, :, 64:65], 1.0)
                nc.gpsimd.memset(V[b][:, :, 129:130], 1.0)

            # phase-D constants: allocated now, DMAs issued after a2a(1)
            wout_sb = const.tile([128, 8, D], BF16, tag="wout")
            boutb_sb = const.tile([128, D], FP32, tag="boutb")
            npad_sb = const.tile([128, TPC // 128], FP32, tag="npad")

            pools2 = ExitStack()
            with tc.tile_pool(name="av_ps", bufs=1, space="PSUM") as avps, \
                 tc.tile_pool(name="sc_ps", bufs=2, space="PSUM") as sps:
                qstack = ExitStack()
                qps = qstack.enter_context(
                    tc.tile_pool(name="qkv_ps", bufs=2, space="PSUM"))

                # ---------- QKV projection, emitted as fill units ----------
                def qkv_units(b):
                    """Yield closures; each emits a chunk of QKV(b).
                    tb=3 first: block (b,3) reads QT columns 1536.. so the
                    last token tile must land before attention starts."""
                    xT_r = xT.rearrange("(fo p) t -> p fo t", p=128)
                    for tb in (3, 0, 1, 2):
                        tw = TW[b][tb]
                        if tw <= 0:
                            continue
                        sl = slice(b * S + tb * 512, b * S + tb * 512 + tw)
                        lsl = slice(tb * 512, tb * 512 + tw)
                        if b == 0 and tb == 3:
                            xt = xt0
                        else:
                            xt = xp.tile([128, 8, 512], BF16, tag="xt",
                                         name="xt")

                            def load(xt=xt, sl=sl, tw=tw, b=b, tb=tb):
                                # b=0 runs upfront: spread its x loads over
                                # three DGE queues
                                if b > 0:
                                    q = nc.sync
                                else:
                                    q = {0: nc.scalar, 1: nc.sync,
                                         2: nc.gpsimd}[tb]
                                q.dma_start(xt[:, :, :tw], xT_r[:, :, sl])
                            yield load

                        def qmm(xt=xt, lsl=lsl, tw=tw):
                            ps = qps.tile([128, 512], FP32, tag="qkv",
                                          name="psq")
                            for fo in range(8):
                                nc.tensor.matmul(ps[:, :tw], wq_sb[:, fo],
                                                 xt[:, fo, :tw],
                                                 start=(fo == 0),
                                                 stop=(fo == 7))
                            nc.scalar.activation(
                                QT[b][:, lsl], ps[:, :tw], IDENT, bias=bq_sb)
                        yield qmm

                        def kmm(xt=xt, lsl=lsl, tw=tw):
                            ps = qps.tile([128, 512], FP32, tag="qkv",
                                          name="psk")
                            for fo in range(8):
                                nc.tensor.matmul(ps[:, :tw], wk_sb[:, fo],
                                                 xt[:, fo, :tw],
                                                 start=(fo == 0),
                                                 stop=(fo == 7))
                            nc.scalar.activation(
                                KT[b][:, lsl], ps[:, :tw], IDENT, bias=bk_sb)
                        yield kmm

                        def vmm(xt=xt, tb=tb, tw=tw, b=b):
                            # V in [vd, tok] orientation, then PE-transpose
                            ps = qps.tile([128, 512], FP32, tag="qkv",
                                          name="psv")
                            for fo in range(8):
                                nc.tensor.matmul(ps[:, :tw], wv_sb[:, fo],
                                                 xt[:, fo, :tw],
                                                 start=(fo == 0),
                                                 stop=(fo == 7))
                            vt_sb = vtp.tile([128, 512], BF16, tag="vt",
                                             name="vt")
                            nc.scalar.activation(
                                vt_sb[:, :tw], ps[:, :tw], IDENT, bias=bv_sb)
                            vtps = qps.tile([128, 4, 128], BF16, tag="qkv",
                                            name="vtps")
                            nt4 = tw // 128
                            for t4 in range(nt4):
                                nc.tensor.transpose(
                                    vtps[:, t4, :],
                                    vt_sb[:, t4 * 128:(t4 + 1) * 128],
                                    idn_sb)
                            c0 = tb * 4
                            nc.vector.tensor_copy(
                                out=V[b][:, c0:c0 + nt4, 0:64],
                                in_=vtps[:, 0:nt4, 0:64])
                            nc.vector.tensor_copy(
                                out=V[b][:, c0:c0 + nt4, 65:129],
                                in_=vtps[:, 0:nt4, 64:128])
                        yield vmm

                # ---------- attention block ----------
                bias_cache = {}
                last_av = [None]

                def attn_block(b, qmb, fill, depth=2, no_gpsimd=False):
                    nkt = min(4 * (qmb + 1), KTC[b])
                    w = QW[b][qmb]
                    dest = b * 2 + qmb // 2
                    hf = qmb % 2
                    avs = [avps.tile([65, 512], FP32, tag=f"av{h}",
                                     name=f"av{h}_{b}_{qmb}")
                           for h in range(HPC)]
                    scs = {}
                    if w <= 0:
                        # no valid q: ship a safe all-ones slot
                        for h in range(HPC):
                            av_sb = ap_.tile([65, 512], BF16, tag=f"avsb{h}")
                            nc.gpsimd.memset(av_sb, 1.0)
                            eng = nc.scalar if h == 0 else nc.gpsimd
                            eng.dma_start(a2a_in[hf][dest][:, h, :], av_sb)
                        return

                    bts = {}

                    def emit_bias(kc):
                        off = max(kc - 4 * qmb, 0) * 128
                        key = (qmb, kc)
                        if qmb == 3:
                            bt = bias_cache.get(key)
                            load_bias = bt is None
                            if load_bias:
                                bt = bcache.tile([128, HPC, 512], BF16,
                                                 tag=f"bt{qmb}_{kc}",
                                                 name=f"bt{qmb}_{kc}")
                                bias_cache[key] = bt
                            bw = 512   # cache must cover every b's width
                        else:
                            bt = bstream.tile([128, HPC, 512], BF16,
                                              tag="bs", name="bs")
                            load_bias = True
                            bw = w
                        if load_bias:
                            # the first block's biases ride the scalar queue
                            # (sync is busy streaming x for QKV(0))
                            bq_ = nc.scalar if (b == 0 and qmb == 3) \
                                else nc.sync
                            bq_.dma_start(
                                bt[:, :, off:bw],
                                ebT[:, kc * 128:(kc + 1) * 128,
                                    qmb * 512 + off:qmb * 512 + bw]
                                .rearrange("h k q -> k h q"))
                        bts[kc] = bt

                    def emit_s(kc):
                        off = max(kc - 4 * qmb, 0) * 128
                        sc = sps.tile([128, HPC, 512], FP32, tag="sc",
                                      name=f"sc_{b}_{qmb}_{kc}")
                        for h in range(HPC):
                            hsl = slice(h * 64, h * 64 + 64)
                            nc.tensor.matmul(
                                sc[:, h, off:w],
                                KT[b][hsl, kc * 128:(kc + 1) * 128],
                                QT[b][hsl, qmb * 512 + off:qmb * 512 + w],
                                start=True, stop=True)
                        scs[kc] = sc

                    for kc in range(min(DB, nkt)):      # bias prefetch
                        emit_bias(kc)
                    for kc in range(min(depth, nkt)):   # score prefetch
                        emit_s(kc)
                    for kc in range(nkt):
                        off = max(kc - 4 * qmb, 0) * 128
                        if kc + DB < nkt:
                            emit_bias(kc + DB)
                        if kc + depth < nkt:
                            emit_s(kc + depth)
                        sc, bt = scs.pop(kc), bts.pop(kc)
                        es = esp.tile([128, HPC, 512], BF16, tag="es")
                        nc.scalar.activation(
                            es[:, :, off:w], sc[:, :, off:w], EXP,
                            bias=pc_col(b, kc))
                        pt = ptp.tile([128, HPC, 512], BF16, tag="pt")
                        for h in range(HPC):
                            eng = (nc.vector if no_gpsimd
                                   or (kc * 2 + h) % 7 < 5
                                   else nc.gpsimd)
                            eng.tensor_tensor(
                                out=pt[:, h, off:w], in0=es[:, h, off:w],
                                in1=bt[:, h, off:w],
                                op=mybir.AluOpType.mult)
                            vsl = slice(h * 65, h * 65 + 65)
                            nc.tensor.matmul(
                                avs[h][:, off:w], V[b][:, kc, vsl],
                                pt[:, h, off:w],
                                start=(kc == 0), stop=(kc == nkt - 1))
                        if fill is not None:
                            u = next(fill, None)
                            if u is not None:
                                u()
                    for h in range(HPC):
                        av_sb = ap_.tile([65, 512], BF16, tag=f"avsb{h}")
                        last_av[0] = av_sb
                        if w < 512:
                            nc.gpsimd.memset(av_sb[:, w:], 1.0)
                        if h == 0:
                            nc.vector.tensor_copy(out=av_sb[:, :w],
                                                  in_=avs[h][:, :w])
                            # hf0: keep av DMAs off the bias-laden sync queue
                            # so the a2a(0) trigger's pooled-semaphore wait
                            # doesn't also cover unrelated bias transfers
                            q = nc.scalar if no_gpsimd else nc.sync
                        else:
                            nc.scalar.activation(av_sb[:, :w],
                                                 avs[h][:, :w], IDENT)
                            q = nc.scalar if no_gpsimd else nc.gpsimd
                        q.dma_start(a2a_in[str(hf)][dest][:, h, :], av_sb)

                def drain(it):
                    if it is not None:
                        for u in it:
                            u()

                def emit_a2a(ch):
                    nc.gpsimd.collective_compute(
                        "AllToAll", mybir.AluOpType.bypass,
                        replica_groups=[list(range(NC_))],
                        ins=[a2a_in[ch][:]], outs=[a2a_out[ch][:]])

                # ---------- phase-D (out-projection) units ----------
                recv = {}
                recvz = {}
                pps = [None]

                def emit_rcv(ch):
                    # The tile scheduler may hoist these DMAs (which WAIT on
                    # the collective semaphore) arbitrarily early in their
                    # queue; a blocked sync/scalar queue head starves the
                    # bias/exp streams, so they ride the gpsimd queue, which
                    # is idle while a collective is in flight.
                    cw = CHW[ch]
                    # Z rows h-major: row = h*8 + i (sel2d matches)
                    recvz[ch] = recvp.tile([2 * NC_, cw], BF16,
                                           tag=f"rzall{ch}",
                                           name=f"rzall{ch}")
                    recv[ch] = recvp.tile([128, NC_, cw], BF16,
                                          tag=f"recv{ch}",
                                          name=f"recv{ch}")
                    for h in range(HPC):
                        nc.gpsimd.dma_start(
                            recvz[ch][h * NC_:(h + 1) * NC_],
                            a2a_out[ch][:, 64, h, :])
                        nc.gpsimd.dma_start(
                            recv[ch][h * 64:(h + 1) * 64],
                            a2a_out[ch][:, 0:64, h, :]
                            .rearrange("i k q -> k i q"))

                def proj_norm(ch, gate=None):
                    cw = CHW[ch]

                    def zprep(ch=ch, cw=cw):
                        # batched reciprocal of all 16 Z rows at once
                        rzf = rzp.tile([2 * NC_, cw], FP32, tag="rzf",
                                       name="rzf")
                        if gate is None:
                            nc.vector.tensor_copy(out=rzf, in_=recvz[ch])
                        else:
                            # x1.0 via the gate tile: a real dependency that
                            # stops the scheduler from hoisting this wait
                            # into the attention-phase vector stream
                            nc.vector.tensor_scalar_mul(
                                rzf, recvz[ch], gate)
                        zr = rzp.tile([2 * NC_, cw], FP32, tag="zr",
                                      name="zr")
                        nc.vector.reciprocal_approx_fast(out=zr, in_=rzf)
                        zrb = rzp.tile([2 * NC_, cw], BF16, tag="zrb",
                                       name="zrb")
                        nc.vector.tensor_copy(out=zrb, in_=zr)
                        recvz[ch] = zrb
                    yield zprep
                    for i in range(NC_):
                        def norm(i=i, ch=ch, cw=cw):
                            bc = pps[0].tile([128, 512], FP32, tag="op",
                                             name="bc")
                            nc.tensor.matmul(bc[:, :cw], sel2[:, i],
                                             recvz[ch],
                                             start=True, stop=True)
                            nc.vector.tensor_tensor(
                                out=recv[ch][:, i], in0=recv[ch][:, i],
                                in1=bc[:, :cw],
                                op=mybir.AluOpType.mult)
                        yield norm

                def proj_otiles(ch, tiles):
                    for tt in tiles:
                        for nb in range(2):
                            def ohalf(tt=tt, nb=nb, ch=ch):
                                ct = (tt % 4) - CH_T0[ch] % 4
                                if nb == 0:
                                    o_sb = op_.tile([128, D], FP32,
                                                    tag="osb",
                                                    name=f"osb{tt}")
                                    proj_osb[tt] = o_sb
                                o_sb = proj_osb[tt]
                                ps = pps[0].tile([128, 512], FP32, tag="op",
                                                 name="ps")
                                for i in range(NC_):
                                    nc.tensor.matmul(
                                        ps,
                                        recv[ch][:, i, ct * 128:
                                                 (ct + 1) * 128],
                                        wout_sb[:, i,
                                                nb * 512:(nb + 1) * 512],
                                        start=(i == 0), stop=(i == NC_ - 1))
                                nsl = slice(nb * 512, (nb + 1) * 512)
                                nc.vector.tensor_tensor(
                                    out=o_sb[:, nsl], in0=ps,
                                    in1=boutb_sb[:, nsl],
                                    op=mybir.AluOpType.add)
                                if nb == 1:
                                    nc.vector.tensor_scalar_mul(
                                        o_sb, o_sb, npad_sb[:, tt:tt + 1])
                                    nc.sync.dma_start(
                                        out[tt * 128:(tt + 1) * 128, :],
                                        o_sb)
                            yield ohalf

                proj_osb = [None] * (TPC // 128)

                def keepalive(n):
                    # scratch matmuls: hold the PE p-state through the
                    # collective wait so the out-projection runs at full rate
                    for i in range(n):
                        ks = pps[0].tile([128, 512], FP32, tag="op",
                                         name="ka")
                        nc.tensor.matmul(
                            ks, wout_sb[:, i % 8, 0:128],
                            QT[0][:, (i % 3) * 512:(i % 3) * 512 + 512],
                            start=True, stop=True)

                # ---------- main pipeline ----------
                for u in qkv_units(0):
                    u()
                # preload block (0,3)'s bias cache behind the b=0 x tiles
                # (first 4 on scalar - needed first; rest on sync) so the
                # first exp chain is never bias-starved
                for kc in range(KTC[0]):
                    bt = bcache.tile([128, HPC, 512], BF16,
                                     tag=f"bt3_{kc}", name=f"bt3_{kc}")
                    bias_cache[(3, kc)] = bt
                    bq_ = nc.scalar if kc < 4 else nc.sync
                    bq_.dma_start(
                        bt, ebT[:, kc * 128:(kc + 1) * 128, 1536:2048]
                        .rearrange("h k q -> k h q"))

                fillgen = {0: qkv_units(1), 2: qkv_units(2), 4: qkv_units(3)}
                cur = None
                for bi, (b, qmb) in enumerate(HF1):
                    cur = fillgen.get(bi, cur)
                    attn_block(b, qmb, fill=cur)
                    if bi in (1, 3, 5):
                        drain(cur)
                        cur = None
                # QKV PSUM banks become phase-D banks
                qstack.close()
                pps[0] = pools2.enter_context(
                    tc.tile_pool(name="proj_ps", bufs=2, space="PSUM"))

                emit_a2a("1")
                emit_rcv("1")
                # phase-D constants (gpsimd queue is blocked by the a2a)
                nc.scalar.dma_start(
                    wout_sb, wout.rearrange("(io p) n -> p io n", p=128))
                nc.scalar.dma_start(boutb_sb, boutb[:])
                nc.scalar.dma_start(npad_sb, npad[:])

                # hf0 runs gpsimd-free: the a2a(1) collective blocks the
                # gpsimd queue for an unpredictable 17-70us, and any gpsimd
                # work queued behind it backlogs the whole softmax chain.
                for bi, (b, qmb) in enumerate(HF0):
                    attn_block(b, qmb, fill=None, no_gpsimd=True)
                # anchor the gate in the last block's av tile: the memset
                # carries a WAR dependency on that tile's a2a-in DMA, which
                # pins everything reading the gate to after attention ends
                gate_src = last_av[0][0:2 * NC_, 0:1]
                nc.vector.memset(gate_src, 1.0)
                gate = rzp.tile([2 * NC_, 1], FP32, tag="gate", name="gate")
                nc.vector.tensor_copy(out=gate, in_=gate_src)

                # a2a(0) first; rcv/norm/proj of half 1 overlap its transfer
                emit_a2a("0")
                emit_rcv("0")
                for u in chain(proj_norm("1", gate=gate),
                               proj_otiles("1", [4, 5, 6, 7])):
                    u()
                keepalive(KA_BRIDGE)
                for u in chain(proj_norm("0"),
                               proj_otiles("0", [0, 1, 2, 3])):
                    u()
                pools2.close()

    nc.finalize()
    return nc


_CACHE: dict = {}


def _prep_inputs(x, Wqkv, bqkv, Wout, bout, causal_mask, rel_bias,
                 key_padding_mask):
    """Host-side shard prep: returns (in_maps, lens)."""
    f32 = np.float32
    bf16 = ml_dtypes.bfloat16
    x = np.asarray(x, f32)
    Wqkv = np.asarray(Wqkv, f32)
    bqkv = np.asarray(bqkv, f32)
    Wout = np.asarray(Wout, f32)
    bout = np.asarray(bout, f32)
    causal_mask = np.asarray(causal_mask, f32)
    rel_bias = np.asarray(rel_bias, f32)
    kpm = np.asarray(key_padding_mask, bool)
    lens = (~kpm).sum(axis=1).astype(np.int64)

    scale = f32(HD ** -0.5)
    xT = np.ascontiguousarray(x.reshape(T, D).T.astype(bf16))

    # key-padding additive column per k-tile: [128, B, KTILES]
    pcm = np.where(kpm, f32(NEG), f32(0.0)).astype(f32)       # [B, S]
    pcm = np.ascontiguousarray(
        pcm.reshape(B, KTILES, 128).transpose(2, 0, 1))       # [128, B, KT]
    boutb = np.ascontiguousarray(np.broadcast_to(bout[None], (128, D)))
    # Z-broadcast selector; Z rows are h-major (row = h*8 + i)
    sel2d = np.zeros((2 * NC_, NC_, 128), f32)
    for i in range(NC_):
        sel2d[i, i, 0:64] = 1.0
        sel2d[NC_ + i, i, 64:128] = 1.0
    sel2d = sel2d.astype(bf16)
    idn = np.eye(128, dtype=bf16)
    notpad_flat = (~kpm).reshape(T).astype(f32)

    pcm_flat = pcm.reshape(128, B * KTILES)
    wout_b = np.ascontiguousarray(Wout.astype(bf16))
    in_maps = []
    for c in range(NC_):
        co = 128 * c
        wq_c = (Wqkv[:, co:co + 128] * scale).astype(bf16)
        wk_c = Wqkv[:, D + co:D + co + 128].astype(bf16)
        wv_c = Wqkv[:, 2 * D + co:2 * D + co + 128].astype(bf16)
        # [p, 3, fo, m]: per-partition contiguous 2KB runs per weight
        wqkv_c = np.ascontiguousarray(
            np.stack([wq_c, wk_c, wv_c], axis=0)
            .reshape(3, 8, 128, 128).transpose(2, 0, 1, 3))
        cvec_c = np.ascontiguousarray(np.concatenate([
            (bqkv[co:co + 128] * scale)[:, None],
            bqkv[D + co:D + co + 128][:, None],
            bqkv[2 * D + co:2 * D + co + 128][:, None],
            pcm_flat], axis=1).astype(f32))
        bias_c = rel_bias[HPC * c:HPC * c + HPC] + causal_mask[None]
        ebT_c = np.ascontiguousarray(
            np.exp(bias_c.transpose(0, 2, 1)).astype(bf16))
        np_c = np.ascontiguousarray(
            notpad_flat[c * TPC:(c + 1) * TPC].reshape(TPC // 128, 128).T)
        in_maps.append({
            "xT": xT, "wqkv": wqkv_c, "cvec": cvec_c,
            "ebT": ebT_c,
            "wout": wout_b,
            "boutb": boutb, "npad": np_c, "sel2d": sel2d, "idn": idn,
        })
    return in_maps, lens


def kernel(**inputs) -> np.ndarray:
    in_maps, lens = _prep_inputs(**inputs)
    key = tuple(int(l) for l in lens)
    if key not in _CACHE:
        _CACHE[key] = build_program(lens)
    nc = _CACHE[key]
    res = run_bass_kernel_spmd(nc, in_maps, core_ids=list(range(NC_)))
    outs = [res.results[c]["out"] for c in range(NC_)]
    return np.concatenate(outs, axis=0).reshape(B, S, D)


# revision 87
# speedup vs baseline: 1.0092x; 1.0092x over previous
"""Trainium2 Bass kernel for 16-head causal attention with relative position
bias (B=4, S=2048, D=1024, H=16, HD=64), distributed over 8 NeuronCores.

Sharding: tensor-parallel over heads - each core owns 2 heads end-to-end
(QKV projection column-sharded, attention, then an on-device AllToAll
re-shards by tokens so each core runs the output projection for a disjoint
1024-token slice). Host only slices weights / concatenates output slices.

v2 restructure (vs the first working kernel):
  - Blocks reordered: all hf=1 (odd-qmb) attention first, so the first
    AllToAll fires at ~60% of attention and overlaps the whole hf=0 phase.
  - The program is JIT-specialized on the actual key-padding lengths
    (read from the inputs at kernel() time): k-tiles and q-columns beyond
    a sequence's length are skipped exactly (their reference contribution
    is identically zero); QKV projection token tiles are trimmed to the
    128-rounded length (~20% less attention work for this seed).
  - V is projected in [vd, tok] orientation with 512-wide matmuls and
    PE-transposed back (the old [tok, vd] form needed 4x the matmul
    instructions at 128-free each).
  - Softmax-normalization broadcast matmul runs in bf16 (the fp32
    LOW_HIGH pairs cost 2.3us each on hw); all 16 Z-reciprocals are
    batched into one DVE pass.
  - Hard-won scheduling rules for the collectives (each violation cost
    15-60us of pipeline stall on hw, invisible in the cost model):
      * A collective's descriptors ride the sync(SP) DGE; ANY queued DMA
        that waits on the collective's semaphore starves the collective
        itself (deadlock until partial drain). All receive-side DMAs
        therefore ride the gpsimd queue, which is idle-by-construction
        while a collective is in flight.
      * No gpsimd compute may be queued behind a collective trigger
        (the queue head blocks until the collective completes), so the
        hf=0 softmax multiplies run vector-only.
      * The tile scheduler hoists collective-dependent work into the
        attention-phase engine streams (its cost model underestimates
        real a2a time 2-4x); the hf=1 Z-normalization is gated on a
        dummy value written into the last attention block's output tile
        so it cannot be hoisted above the end of attention.
  - Consolidated input layouts (packed wqkv, one constant vector) to kill
    256B/4B-descriptor DMAs that serialized ~15us of startup.

Compute dtype: bf16 matmul inputs, fp32 logits/accumulation.
Rel. error vs fp32 reference: 6.1e-3 (bit-identical across runs; the
original kernel showed trace-dependent corruption up to 2.2e-2).
"""

from contextlib import ExitStack
from itertools import chain

import numpy as np
import ml_dtypes

import concourse.bass as bass
import concourse.mybir as mybir
from concourse import bacc
from concourse.tile import TileContext
from concourse.bass_utils import run_bass_kernel_spmd

B, S, D, H = 4, 2048, 1024, 16
HD = D // H                  # 64
NC_ = 8                      # cores
HPC = H // NC_               # 2 heads per core
T = B * S                    # 8192 tokens
TPC = T // NC_               # 1024 tokens per core (out-proj shard)
NEG = -1e9
FP32 = mybir.dt.float32
BF16 = mybir.dt.bfloat16

KTILES = S // 128            # 16 k-tiles per sequence
IDENT = mybir.ActivationFunctionType.Identity
EXP = mybir.ActivationFunctionType.Exp

HF1 = [(0, 3), (0, 1), (1, 3), (1, 1), (2, 3), (2, 1), (3, 3), (3, 1)]
HF0 = [(0, 0), (0, 2), (1, 0), (1, 2), (2, 0), (2, 2), (3, 0), (3, 2)]
KA_BRIDGE = 0    # keepalive matmuls bridging the a2a(0) wait (throttled
                 # PE gains nothing from p-state keepalives; keep 0)
DB = 6           # bias-tile DMA prefetch depth (absorbs sync-queue stalls)


def build_program(lens) -> bass.Bass:
    """Build the (identical-on-every-core) SPMD Bass program, specialized
    to the per-batch valid lengths `lens` (from key_padding_mask)."""
    lens = [int(l) for l in lens]
    KTC = [(l + 127) // 128 for l in lens]           # valid k-tiles per b
    L128 = [ktc * 128 for ktc in KTC]                # 128-rounded lengths
    # qkv token-tile widths (128-rounded so K/V rows stay finite)
    TW = [[max(0, min(512, L128[b] - tb * 512)) for tb in range(4)]
          for b in range(B)]
    # exact q widths per 512-block
    QW = [[max(0, min(512, lens[b] - qmb * 512)) for qmb in range(4)]
          for b in range(B)]

    nc = bacc.Bacc(num_devices=NC_)

    # ---- I/O ----
    xT = nc.dram_tensor("xT", [D, T], BF16, kind="ExternalInput")
    # wqkv pre-arranged on host: [p, (q|k|v), fo, m] so each partition row
    # is one contiguous 2KB-per-weight run (256B descriptors cost 2x)
    wqkv = nc.dram_tensor("wqkv", [128, 3, 8, 128], BF16,
                          kind="ExternalInput")
    # cvec: col 0 bq, 1 bk, 2 bv, 3.. pc (B*KTILES)
    cvec = nc.dram_tensor("cvec", [128, 3 + B * KTILES], FP32,
                          kind="ExternalInput")
    # transposed multiplicative bias: ebT[h, k, q] =
    #   exp(rel_bias[h, q, k] + causal[q, k])  (exactly 0 where masked)
    ebT = nc.dram_tensor("ebT", [HPC, S, S], BF16, kind="ExternalInput")
    wout = nc.dram_tensor("wout", [D, D], BF16, kind="ExternalInput")
    boutb = nc.dram_tensor("boutb", [128, D], FP32, kind="ExternalInput")
    npad = nc.dram_tensor("npad", [128, TPC // 128], FP32, kind="ExternalInput")
    # sel2d[r, i, p] = 1 iff r == 2*i + (p >= 64): per-source Z broadcaster
    sel2d = nc.dram_tensor("sel2d", [2 * NC_, NC_, 128], BF16,
                           kind="ExternalInput")
    idn = nc.dram_tensor("idn", [128, 128], BF16, kind="ExternalInput")
    out = nc.dram_tensor("out", [TPC, D], FP32, kind="ExternalOutput")

    with TileContext(nc) as tc:
        with tc.tile_pool(name="const", bufs=1) as const, \
             tc.tile_pool(name="dram", bufs=1, space="DRAM") as dpool, \
             tc.tile_pool(name="big", bufs=1) as big, \
             tc.tile_pool(name="xp", bufs=2) as xp, \
             tc.tile_pool(name="vtp", bufs=2) as vtp, \
             tc.tile_pool(name="bcache", bufs=1) as bcache, \
             tc.tile_pool(name="bstream", bufs=8) as bstream, \
             tc.tile_pool(name="esp", bufs=3) as esp, \
             tc.tile_pool(name="ptp", bufs=3) as ptp, \
             tc.tile_pool(name="ap_", bufs=2) as ap_, \
             tc.tile_pool(name="recvp", bufs=1) as recvp, \
             tc.tile_pool(name="rzp", bufs=2) as rzp, \
             tc.tile_pool(name="op_", bufs=2) as op_:

            # ---- constants: first-use loads split across all 3 DGE queues
            # preload the tb=3 x tile of b=0 (first consumed: block (0,3))
            xt0 = xp.tile([128, 8, 512], BF16, tag="xt", name="xt0")
            xT_r0 = xT.rearrange("(fo p) t -> p fo t", p=128)
            tw0 = TW[0][3]
            wqkv_sb = const.tile([128, 3, 8, 128], BF16, tag="wqkv")
            nc.gpsimd.dma_start(wqkv_sb, wqkv[:])
            if tw0 > 0:
                nc.sync.dma_start(xt0[:, :, :tw0],
                                  xT_r0[:, :, 1536:1536 + tw0])
            wq_sb, wk_sb, wv_sb = (wqkv_sb[:, i] for i in range(3))
            cvec_sb = const.tile([128, 3 + B * KTILES], FP32, tag="cvec")
            nc.sync.dma_start(cvec_sb, cvec[:])
            bq_sb = cvec_sb[:, 0:1]
            bk_sb = cvec_sb[:, 1:2]
            bv_sb = cvec_sb[:, 2:3]

            def pc_col(b, kc):
                i = 3 + b * KTILES + kc
                return cvec_sb[:, i:i + 1]
            # selector for the per-head Z broadcast: row h -> out rows h*64..
            sel2 = const.tile([2 * NC_, NC_, 128], BF16, tag="sel2")
            nc.sync.dma_start(sel2, sel2d[:])
            idn_sb = const.tile([128, 128], BF16, tag="idn")
            nc.sync.dma_start(idn_sb, idn[:])

            # ---- internal DRAM for the AllToAlls ----
            # channels: "1" = whole hf=1 half (fully overlapped by hf=0
            # attention); "0a"/"0b" = column-halves of hf=0, pipelined so
            # the first half's projection overlaps the second's transfer.
            CHW = {"1": TPC // 2, "0": TPC // 2}
            CH_T0 = {"1": 4, "0": 0}   # first out token-tile
            a2a_in = {c: dpool.tile([NC_, 65, HPC, w], BF16,
                                    tag=f"a2a_in{c}", name=f"a2a_in{c}")
                      for c, w in CHW.items()}
            a2a_out = {c: dpool.tile([NC_, 65, HPC, w], BF16,
                                     tag=f"a2a_out{c}", name=f"a2a_out{c}")
                       for c, w in CHW.items()}

            # ---- persistent per-b intermediates ----
            # QT/KT: [2*HD qdims (h0 0:64, h1 64:128), S tokens]
            QT = [big.tile([128, S], BF16, tag=f"QT{b}", name=f"QT{b}")
                  for b in range(B)]
            KT = [big.tile([128, S], BF16, tag=f"KT{b}", name=f"KT{b}")
                  for b in range(B)]
            # V: [128 token-part, 16 token-chunks, 130]:
            #   cols 0:64 head0, 64 ones, 65:129 head1, 129 ones
            V = [big.tile([128, KTILES, 130], BF16, tag=f"V{b}", name=f"V{b}")
                 for b in range(B)]
            for b in range(B):
                nc.gpsimd.memset(V[b][:, :, 64:65], 1.0)
                nc.gpsimd.memset(V[b][:, :, 129:130], 1.0)

            # phase-D constants: allocated now, DMAs issued after a2a(1)
            wout_sb = const.tile([128, 8, D], BF16, tag="wout")
            boutb_sb = const.tile([128, D], FP32, tag="boutb")
            npad_sb = const.tile([128, TPC // 128], FP32, tag="npad")

            pools2 = ExitStack()
            with tc.tile_pool(name="av_ps", bufs=1, space="PSUM") as avps, \
                 tc.tile_pool(name="sc_ps", bufs=2, space="PSUM") as sps:
                qstack = ExitStack()
                qps = qstack.enter_context(
                    tc.tile_pool(name="qkv_ps", bufs=2, space="PSUM"))

                # ---------- QKV projection, emitted as fill units ----------
                def qkv_units(b):
                    """Yield closures; each emits a chunk of QKV(b).
                    tb=3 first: block (b,3) reads QT columns 1536.. so the
                    last token tile must land before attention starts."""
                    xT_r = xT.rearrange("(fo p) t -> p fo t", p=128)
                    for tb in (3, 0, 1, 2):
                        tw = TW[b][tb]
                        if tw <= 0:
                            continue
                        sl = slice(b * S + tb * 512, b * S + tb * 512 + tw)
                        lsl = slice(tb * 512, tb * 512 + tw)
                        if b == 0 and tb == 3:
                            xt = xt0
                        else:
                            xt = xp.tile([128, 8, 512], BF16, tag="xt",
                                         name="xt")

                            def load(xt=xt, sl=sl, tw=tw, b=b, tb=tb):
                                # b=0 runs upfront: spread its x loads over
                                # three DGE queues
                                if b > 0:
                                    q = nc.sync
                                else:
                                    q = {0: nc.scalar, 1: nc.sync,
                                         2: nc.gpsimd}[tb]
                                q.dma_start(xt[:, :, :tw], xT_r[:, :, sl])
                            yield load

                        def qmm(xt=xt, lsl=lsl, tw=tw):
                            ps = qps.tile([128, 512], FP32, tag="qkv",
                                          name="psq")
                            for fo in range(8):
                                nc.tensor.matmul(ps[:, :tw], wq_sb[:, fo],
                                                 xt[:, fo, :tw],
                                                 start=(fo == 0),
                                                 stop=(fo == 7))
                            nc.scalar.activation(
                                QT[b][:, lsl], ps[:, :tw], IDENT, bias=bq_sb)
                        yield qmm

                        def kmm(xt=xt, lsl=lsl, tw=tw):
                            ps = qps.tile([128, 512], FP32, tag="qkv",
                                          name="psk")
                            for fo in range(8):
                                nc.tensor.matmul(ps[:, :tw], wk_sb[:, fo],
                                                 xt[:, fo, :tw],
                                                 start=(fo == 0),
                                                 stop=(fo == 7))
                            nc.scalar.activation(
                                KT[b][:, lsl], ps[:, :tw], IDENT, bias=bk_sb)
                        yield kmm

                        def vmm(xt=xt, tb=tb, tw=tw, b=b):
                            # V in [vd, tok] orientation, then PE-transpose
                            ps = qps.tile([128, 512], FP32, tag="qkv",
                                          name="psv")
                            for fo in range(8):
                                nc.tensor.matmul(ps[:, :tw], wv_sb[:, fo],
                                                 xt[:, fo, :tw],
                                                 start=(fo == 0),
                                                 stop=(fo == 7))
                            vt_sb = vtp.tile([128, 512], BF16, tag="vt",
                                             name="vt")
                            nc.scalar.activation(
                                vt_sb[:, :tw], ps[:, :tw], IDENT, bias=bv_sb)
                            vtps = qps.tile([128, 4, 128], BF16, tag="qkv",
                                            name="vtps")
                            nt4 = tw // 128
                            for t4 in range(nt4):
                                nc.tensor.transpose(
                                    vtps[:, t4, :],
                                    vt_sb[:, t4 * 128:(t4 + 1) * 128],
                                    idn_sb)
                            c0 = tb * 4
                            nc.vector.tensor_copy(
                                out=V[b][:, c0:c0 + nt4, 0:64],
                                in_=vtps[:, 0:nt4, 0:64])
                            nc.vector.tensor_copy(
                                out=V[b][:, c0:c0 + nt4, 65:129],
                                in_=vtps[:, 0:nt4, 64:128])
                        yield vmm

                # ---------- attention block ----------
                bias_cache = {}
                last_av = [None]

                def attn_block(b, qmb, fill, depth=2, no_gpsimd=False):
                    nkt = min(4 * (qmb + 1), KTC[b])
                    w = QW[b][qmb]
                    dest = b * 2 + qmb // 2
                    hf = qmb % 2
                    avs = [avps.tile([65, 512], FP32, tag=f"av{h}",
                                     name=f"av{h}_{b}_{qmb}")
                           for h in range(HPC)]
                    scs = {}
                    if w <= 0:
                        # no valid q: ship a safe all-ones slot
                        for h in range(HPC):
                            av_sb = ap_.tile([65, 512], BF16, tag=f"avsb{h}")
                            nc.gpsimd.memset(av_sb, 1.0)
                            eng = nc.scalar if h == 0 else nc.gpsimd
                            eng.dma_start(a2a_in[hf][dest][:, h, :], av_sb)
                        return

                    bts = {}

                    def emit_bias(kc):
                        off = max(kc - 4 * qmb, 0) * 128
                        key = (qmb, kc)
                        if qmb == 3:
                            bt = bias_cache.get(key)
                            load_bias = bt is None
                            if load_bias:
                                bt = bcache.tile([128, HPC, 512], BF16,
                                                 tag=f"bt{qmb}_{kc}",
                                                 name=f"bt{qmb}_{kc}")
                                bias_cache[key] = bt
                            bw = 512   # cache must cover every b's width
                        else:
                            bt = bstream.tile([128, HPC, 512], BF16,
                                              tag="bs", name="bs")
                            load_bias = True
                            bw = w
                        if load_bias:
                            # the first block's biases ride the scalar queue
                            # (sync is busy streaming x for QKV(0))
                            bq_ = nc.scalar if (b == 0 and qmb == 3) \
                                else nc.sync
                            bq_.dma_start(
                                bt[:, :, off:bw],
                                ebT[:, kc * 128:(kc + 1) * 128,
                                    qmb * 512 + off:qmb * 512 + bw]
                                .rearrange("h k q -> k h q"))
                        bts[kc] = bt

                    def emit_s(kc):
                        off = max(kc - 4 * qmb, 0) * 128
                        sc = sps.tile([128, HPC, 512], FP32, tag="sc",
                                      name=f"sc_{b}_{qmb}_{kc}")
                        for h in range(HPC):
                            hsl = slice(h * 64, h * 64 + 64)
                            nc.tensor.matmul(
                                sc[:, h, off:w],
                                KT[b][hsl, kc * 128:(kc + 1) * 128],
                                QT[b][hsl, qmb * 512 + off:qmb * 512 + w],
                                start=True, stop=True)
                        scs[kc] = sc

                    for kc in range(min(DB, nkt)):      # bias prefetch
                        emit_bias(kc)
                    for kc in range(min(depth, nkt)):   # score prefetch
                        emit_s(kc)
                    for kc in range(nkt):
                        off = max(kc - 4 * qmb, 0) * 128
                        if kc + DB < nkt:
                            emit_bias(kc + DB)
                        if kc + depth < nkt:
                            emit_s(kc + depth)
                        sc, bt = scs.pop(kc), bts.pop(kc)
                        es = esp.tile([128, HPC, 512], BF16, tag="es")
                        nc.scalar.activation(
                            es[:, :, off:w], sc[:, :, off:w], EXP,
                            bias=pc_col(b, kc))
                        pt = ptp.tile([128, HPC, 512], BF16, tag="pt")
                        for h in range(HPC):
                            eng = (nc.vector if no_gpsimd
                                   or (kc * 2 + h) % 7 < 5
                                   else nc.gpsimd)
                            eng.tensor_tensor(
                                out=pt[:, h, off:w], in0=es[:, h, off:w],
                                in1=bt[:, h, off:w],
                                op=mybir.AluOpType.mult)
                            vsl = slice(h * 65, h * 65 + 65)
                            nc.tensor.matmul(
                                avs[h][:, off:w], V[b][:, kc, vsl],
                                pt[:, h, off:w],
                                start=(kc == 0), stop=(kc == nkt - 1))
                        if fill is not None:
                            u = next(fill, None)
                            if u is not None:
                                u()
                    for h in range(HPC):
                        av_sb = ap_.tile([65, 512], BF16, tag=f"avsb{h}")
                        last_av[0] = av_sb
                        if w < 512:
                            nc.gpsimd.memset(av_sb[:, w:], 1.0)
                        if h == 0:
                            nc.vector.tensor_copy(out=av_sb[:, :w],
                                                  in_=avs[h][:, :w])
                            # hf0: keep av DMAs off the bias-laden sync queue
                            # so the a2a(0) trigger's pooled-semaphore wait
                            # doesn't also cover unrelated bias transfers
                            q = nc.scalar if no_gpsimd else nc.sync
                        else:
                            nc.scalar.activation(av_sb[:, :w],
                                                 avs[h][:, :w], IDENT)
                            q = nc.scalar if no_gpsimd else nc.gpsimd
                        q.dma_start(a2a_in[str(hf)][dest][:, h, :], av_sb)

                def drain(it):
                    if it is not None:
                        for u in it:
                            u()

                def emit_a2a(ch):
                    nc.gpsimd.collective_compute(
                        "AllToAll", mybir.AluOpType.bypass,
                        replica_groups=[list(range(NC_))],
                        ins=[a2a_in[ch][:]], outs=[a2a_out[ch][:]])

                # ---------- phase-D (out-projection) units ----------
                recv = {}
                recvz = {}
                pps = [None]

                def emit_rcv(ch):
                    # The tile scheduler may hoist these DMAs (which WAIT on
                    # the collective semaphore) arbitrarily early in their
                    # queue; a blocked sync/scalar queue head starves the
                    # bias/exp streams, so they ride the gpsimd queue, which
                    # is idle while a collective is in flight.
                    cw = CHW[ch]
                    # Z rows h-major: row = h*8 + i (sel2d matches)
                    recvz[ch] = recvp.tile([2 * NC_, cw], BF16,
                                           tag=f"rzall{ch}",
                                           name=f"rzall{ch}")
                    recv[ch] = recvp.tile([128, NC_, cw], BF16,
                                          tag=f"recv{ch}",
                                          name=f"recv{ch}")
                    for h in range(HPC):
                        nc.gpsimd.dma_start(
                            recvz[ch][h * NC_:(h + 1) * NC_],
                            a2a_out[ch][:, 64, h, :])
                        nc.gpsimd.dma_start(
                            recv[ch][h * 64:(h + 1) * 64],
                            a2a_out[ch][:, 0:64, h, :]
                            .rearrange("i k q -> k i q"))

                def proj_norm(ch, gate=None):
                    cw = CHW[ch]

                    def zprep(ch=ch, cw=cw):
                        # batched reciprocal of all 16 Z rows at once
                        rzf = rzp.tile([2 * NC_, cw], FP32, tag="rzf",
                                       name="rzf")
                        if gate is None:
                            nc.vector.tensor_copy(out=rzf, in_=recvz[ch])
                        else:
                            # x1.0 via the gate tile: a real dependency that
                            # stops the scheduler from hoisting this wait
                            # into the attention-phase vector stream
                            nc.vector.tensor_scalar_mul(
                                rzf, recvz[ch], gate)
                        zr = rzp.tile([2 * NC_, cw], FP32, tag="zr",
                                      name="zr")
                        nc.vector.reciprocal_approx_fast(out=zr, in_=rzf)
                        zrb = rzp.tile([2 * NC_, cw], BF16, tag="zrb",
                                       name="zrb")
                        nc.vector.tensor_copy(out=zrb, in_=zr)
                        recvz[ch] = zrb
                    yield zprep
                    for i in range(NC_):
                        def norm(i=i, ch=ch, cw=cw):
                            bc = pps[0].tile([128, 512], FP32, tag="op",
                                             name="bc")
                            nc.tensor.matmul(bc[:, :cw], sel2[:, i],
                                             recvz[ch],
                                             start=True, stop=True)
                            nc.vector.tensor_tensor(
                                out=recv[ch][:, i], in0=recv[ch][:, i],
                                in1=bc[:, :cw],
                                op=mybir.AluOpType.mult)
                        yield norm

                def proj_otiles(ch, tiles):
                    for tt in tiles:
                        for nb in range(2):
                            def ohalf(tt=tt, nb=nb, ch=ch):
                                ct = (tt % 4) - CH_T0[ch] % 4
                                if nb == 0:
                                    o_sb = op_.tile([128, D], FP32,
                                                    tag="osb",
                                                    name=f"osb{tt}")
                                    proj_osb[tt] = o_sb
                                o_sb = proj_osb[tt]
                                ps = pps[0].tile([128, 512], FP32, tag="op",
                                                 name="ps")
                                for i in range(NC_):
                                    nc.tensor.matmul(
                                        ps,
                                        recv[ch][:, i, ct * 128:
                                                 (ct + 1) * 128],
                                        wout_sb[:, i,
                                                nb * 512:(nb + 1) * 512],
                                        start=(i == 0), stop=(i == NC_ - 1))
                                nsl = slice(nb * 512, (nb + 1) * 512)
                                nc.vector.tensor_tensor(
                                    out=o_sb[:, nsl], in0=ps,
                                    in1=boutb_sb[:, nsl],
                                    op=mybir.AluOpType.add)
                                if nb == 1:
                                    nc.vector.tensor_scalar_mul(
                                        o_sb, o_sb, npad_sb[:, tt:tt + 1])
                                    nc.sync.dma_start(
                                        out[tt * 128:(tt + 1) * 128, :],
                                        o_sb)
                            yield ohalf

                proj_osb = [None] * (TPC // 128)

                def keepalive(n):
                    # scratch matmuls: hold the PE p-state through the
                    # collective wait so the out-projection runs at full rate
                    for i in range(n):
                        ks = pps[0].tile([128, 512], FP32, tag="op",
                                         name="ka")
                        nc.tensor.matmul(
                            ks, wout_sb[:, i % 8, 0:128],
                            QT[0][:, (i % 3) * 512:(i % 3) * 512 + 512],
                            start=True, stop=True)

                # ---------- main pipeline ----------
                # preload block (0,3)'s bias cache on the sync queue before
                # the x tiles so the first exp chain is never bias-starved
                for kc in range(KTC[0]):
                    bt = bcache.tile([128, HPC, 512], BF16,
                                     tag=f"bt3_{kc}", name=f"bt3_{kc}")
                    bias_cache[(3, kc)] = bt
                    nc.sync.dma_start(
                        bt, ebT[:, kc * 128:(kc + 1) * 128, 1536:2048]
                        .rearrange("h k q -> k h q"))

                for u in qkv_units(0):
                    u()

                fillgen = {0: qkv_units(1), 2: qkv_units(2), 4: qkv_units(3)}
                cur = None
                for bi, (b, qmb) in enumerate(HF1):
                    cur = fillgen.get(bi, cur)
                    attn_block(b, qmb, fill=cur)
                    if bi in (1, 3, 5):
                        drain(cur)
                        cur = None
                # QKV PSUM banks become phase-D banks
                qstack.close()
                pps[0] = pools2.enter_context(
                    tc.tile_pool(name="proj_ps", bufs=2, space="PSUM"))

                emit_a2a("1")
                emit_rcv("1")
                # phase-D constants (gpsimd queue is blocked by the a2a)
                nc.scalar.dma_start(
                    wout_sb, wout.rearrange("(io p) n -> p io n", p=128))
                nc.scalar.dma_start(boutb_sb, boutb[:])
                nc.scalar.dma_start(npad_sb, npad[:])

                # hf0 runs gpsimd-free: the a2a(1) collective blocks the
                # gpsimd queue for an unpredictable 17-70us, and any gpsimd
                # work queued behind it backlogs the whole softmax chain.
                for bi, (b, qmb) in enumerate(HF0):
                    attn_block(b, qmb, fill=None, no_gpsimd=True)
                # anchor the gate in the last block's av tile: the memset
                # carries a WAR dependency on that tile's a2a-in DMA, which
                # pins everything reading the gate to after attention ends
                gate_src = last_av[0][0:2 * NC_, 0:1]
                nc.vector.memset(gate_src, 1.0)
                gate = rzp.tile([2 * NC_, 1], FP32, tag="gate", name="gate")
                nc.vector.tensor_copy(out=gate, in_=gate_src)

                # a2a(0) first; rcv/norm/proj of half 1 overlap its transfer
                emit_a2a("0")
                emit_rcv("0")
                for u in chain(proj_norm("1", gate=gate),
                               proj_otiles("1", [4, 5, 6, 7])):
                    u()
                keepalive(KA_BRIDGE)
                for u in chain(proj_norm("0"),
                               proj_otiles("0", [0, 1, 2, 3])):
                    u()
                pools2.close()

    nc.finalize()
    return nc


_CACHE: dict = {}


def _prep_inputs(x, Wqkv, bqkv, Wout, bout, causal_mask, rel_bias,
                 key_padding_mask):
    """Host-side shard prep: returns (in_maps, lens)."""
    f32 = np.float32
    bf16 = ml_dtypes.bfloat16
    x = np.asarray(x, f32)
    Wqkv = np.asarray(Wqkv, f32)
    bqkv = np.asarray(bqkv, f32)
    Wout = np.asarray(Wout, f32)
    bout = np.asarray(bout, f32)
    causal_mask = np.asarray(causal_mask, f32)
    rel_bias = np.asarray(rel_bias, f32)
    kpm = np.asarray(key_padding_mask, bool)
    lens = (~kpm).sum(axis=1).astype(np.int64)

    scale = f32(HD ** -0.5)
    xT = np.ascontiguousarray(x.reshape(T, D).T.astype(bf16))

    # key-padding additive column per k-tile: [128, B, KTILES]
    pcm = np.where(kpm, f32(NEG), f32(0.0)).astype(f32)       # [B, S]
    pcm = np.ascontiguousarray(
        pcm.reshape(B, KTILES, 128).transpose(2, 0, 1))       # [128, B, KT]
    boutb = np.ascontiguousarray(np.broadcast_to(bout[None], (128, D)))
    # Z-broadcast selector; Z rows are h-major (row = h*8 + i)
    sel2d = np.zeros((2 * NC_, NC_, 128), f32)
    for i in range(NC_):
        sel2d[i, i, 0:64] = 1.0
        sel2d[NC_ + i, i, 64:128] = 1.0
    sel2d = sel2d.astype(bf16)
    idn = np.eye(128, dtype=bf16)
    notpad_flat = (~kpm).reshape(T).astype(f32)

    pcm_flat = pcm.reshape(128, B * KTILES)
    wout_b = np.ascontiguousarray(Wout.astype(bf16))
    in_maps = []
    for c in range(NC_):
        co = 128 * c
        wq_c = (Wqkv[:, co:co + 128] * scale).astype(bf16)
        wk_c = Wqkv[:, D + co:D + co + 128].astype(bf16)
        wv_c = Wqkv[:, 2 * D + co:2 * D + co + 128].astype(bf16)
        # [p, 3, fo, m]: per-partition contiguous 2KB runs per weight
        wqkv_c = np.ascontiguousarray(
            np.stack([wq_c, wk_c, wv_c], axis=0)
            .reshape(3, 8, 128, 128).transpose(2, 0, 1, 3))
        cvec_c = np.ascontiguousarray(np.concatenate([
            (bqkv[co:co + 128] * scale)[:, None],
            bqkv[D + co:D + co + 128][:, None],
            bqkv[2 * D + co:2 * D + co + 128][:, None],
            pcm_flat], axis=1).astype(f32))
        bias_c = rel_bias[HPC * c:HPC * c + HPC] + causal_mask[None]
        ebT_c = np.ascontiguousarray(
            np.exp(bias_c.transpose(0, 2, 1)).astype(bf16))
        np_c = np.ascontiguousarray(
            notpad_flat[c * TPC:(c + 1) * TPC].reshape(TPC // 128, 128).T)
        in_maps.append({
            "xT": xT, "wqkv": wqkv_c, "cvec": cvec_c,
            "ebT": ebT_c,
            "wout": wout_b,
            "boutb": boutb, "npad": np_c, "sel2d": sel2d, "idn": idn,
        })
    return in_maps, lens


def kernel(**inputs) -> np.ndarray:
    in_maps, lens = _prep_inputs(**inputs)
    key = tuple(int(l) for l in lens)
    if key not in _CACHE:
        _CACHE[key] = build_program(lens)
    nc = _CACHE[key]
    res = run_bass_kernel_spmd(nc, in_maps, core_ids=list(range(NC_)))
    outs = [res.results[c]["out"] for c in range(NC_)]
    return np.concatenate(outs, axis=0).reshape(B, S, D)
